# revision 1
# baseline (speedup 1.0000x reference)
"""Trainium2 Bass kernel for the inverse deep-hough-transform gather-reduce.

out[n, c, y, x] = sum_k acc[n, c, k, rho_idx[k, y, x]]  (masked by validity)

Design (v2)
-----------
- The rho index table is a pure function of static shapes; precomputed on the
  host.  Validity is folded in by pointing invalid entries at a zeroed spare
  slot (block R) of the data table.
- Gather primitive: GPSIMD IndirectCopy.  Measured cost is per 3-index
  read-request (~102-cycle serial RD_CMD), so each index fetches a contiguous
  block of F=32 nc-values -> 3.6us per 32-index IC (1024 elems/partition, the
  ISA cap per IC).
- Sharding: every core holds ALL 512 (n,c) rows; angles are sharded as
  k === core (mod 8).  Host sums the 8 per-core partials (the unshard step).
- Layout: 128 partitions = 8 groups x 16.  At step s, group g processes angle
  slot s*8+g (3 steps x 8 groups = 24 slots >= 23 angles/core; extra slots
  gather zeros).  Partition p = g*16+u plus block lane f in [0,32) covers
  nc = f*16+u.
- Per yx-chunk (64 positions): GPSIMD gathers [128, 2048] per step, DVE
  accumulates over steps, PE reduces the 8 groups with an exact 0/1 fp32
  selection matmul into PSUM, ACT copies PSUM->SBUF, sync DMA dumps to HBM.
- Raw Bass blocks with explicit semaphores (this walrus build allows at most
  one attached sync-wait per engine instruction; standalone EVSEM waits are
  used instead).
"""

from contextlib import ExitStack

import numpy as np

import concourse.bass as bass
from concourse import mybir
from concourse.bass_utils import run_bass_kernel_spmd

# Problem constants (hardcoded per the harness contract).
N, C, A, R = 4, 128, 180, 184
H = W = 128
YX = H * W  # 16384
NCORES = 8

GROUPS = 8  # 16-partition groups
U = 16  # partitions per group
F = 32  # nc values packed per rho block (IC inner size)
NCPC = F * U  # 512 nc rows held per core (all of them)
STEPS = 3  # angle slots per core = STEPS*GROUPS = 24 >= ceil(180/8)
ASLOT = STEPS * GROUPS  # 24
RPAD = R + 1  # 185 blocks; block R is all-zeros (invalid sink)
CHUNK = 64  # yx positions per chunk
NCH = YX // CHUNK  # 256 chunks
DW = RPAD * F  # data words per step per partition (5920)
CW = CHUNK // U  # idx columns per (step, chunk) per partition (4)
ICSUB = 1024 // F  # indices per IC (32): ISA caps IC dst at 1024 elems
NSUB = CHUNK // ICSUB  # sub-ICs per (chunk, step) (2)

_cache = {}


def _core_angles(core):
    """Angle slots for this core: slot t (0..23) -> global k or None."""
    ks = [k for k in range(A) if k % NCORES == core]
    return [ks[t] if t < len(ks) else None for t in range(ASLOT)]


def _rho_block_table():
    """[A, YX] int32 block indices into the padded rho axis (R = invalid)."""
    if "blk" in _cache:
        return _cache["blk"]
    k = np.arange(A)
    theta = k * (np.pi / A)
    cos_t = np.cos(theta)
    sin_t = np.sin(theta)
    y, x = np.meshgrid(np.arange(H), np.arange(W), indexing="ij")
    xc = (x - W // 2).astype(np.float64)
    yc = (y - H // 2).astype(np.float64)
    r = np.round(cos_t[:, None, None] * xc[None] + sin_t[:, None, None] * yc[None])
    r = r.astype(np.int64) + R // 2  # [A, H, W]
    valid = (r >= 0) & (r < R)
    blk = np.where(valid, np.clip(r, 0, R - 1), R).astype(np.int32)
    _cache["blk"] = blk.reshape(A, YX)
    return _cache["blk"]


def _idx_table(core):
    """uint16 idx stream for one core, SBUF layout [128, STEPS*NCH*CW].

    IndirectCopy unwraps a 16-partition group's idx tile as
    (col*16 + p_in_group); values are flat element offsets = block*F.
    Column layout: col = s*(NCH*CW) + q*CW + w.
    """
    key = ("idx", core)
    if key in _cache:
        return _cache[key]
    blk = _rho_block_table()
    angles = _core_angles(core)
    out = np.full((128, STEPS * NCH * CW), R * F, np.uint16)
    for s in range(STEPS):
        for g in range(GROUPS):
            k = angles[s * GROUPS + g]
            if k is None:
                continue
            flat = (blk[k] * F).astype(np.uint16)  # [YX]
            v = flat.reshape(NCH, CW, U)  # [q, w, p_in_group]
            v = v.transpose(2, 0, 1).reshape(U, NCH * CW)
            cols = slice(s * NCH * CW, (s + 1) * NCH * CW)
            out[g * U : (g + 1) * U, cols] = v
    _cache[key] = out
    return out


def _data_for_core(acc_flat, core):
    """acc_flat [512, A, R] f32 -> [128, STEPS*DW] f32 padded gather table.

    data[g*16+u, s*DW + rb*F + f] = acc_flat[f*16+u, k(s*8+g), rb]
    """
    angles = _core_angles(core)
    ac = np.zeros((NCPC, ASLOT, RPAD), np.float32)
    for t, k in enumerate(angles):
        if k is not None:
            ac[:, t, :R] = acc_flat[:, k, :]
    t = ac.reshape(F, U, ASLOT, RPAD)  # [f, u, t, r]
    t = t.transpose(2, 1, 3, 0)  # [t, u, r, f]
    t = t.reshape(STEPS, GROUPS, U, RPAD, F)  # [s, g, u, r, f]
    t = t.reshape(STEPS, 128, DW)
    return np.ascontiguousarray(t.transpose(1, 0, 2).reshape(128, STEPS * DW))


def _sel_matrix():
    """[128, 16] f32 selection: S[p, m] = 1 if p % 16 == m (group reduce)."""
    s = np.zeros((128, U), np.float32)
    s[np.arange(128), np.arange(128) % U] = 1.0
    return s


def _build_nc():
    if "nc" in _cache:
        return _cache["nc"]
    nc = bass.Bass("TRN2", debug=False, target_bir_lowering=False, num_devices=NCORES)
    data_d = nc.dram_tensor(
        "data", [128, STEPS * DW], mybir.dt.float32, kind="ExternalInput"
    ).ap()
    idx_d = nc.dram_tensor(
        "idx", [128, STEPS * NCH * CW], mybir.dt.uint16, kind="ExternalInput"
    ).ap()
    sel_d = nc.dram_tensor(
        "sel", [128, U], mybir.dt.float32, kind="ExternalInput"
    ).ap()
    raw_d = nc.dram_tensor(
        "raw", [NCH, U, CHUNK * F], mybir.dt.float32, kind="ExternalOutput"
    ).ap()

    GW = CHUNK * F  # 2048 gather/accum words per partition per (chunk, step)
    NMM = GW // 512  # matmuls per chunk (PSUM bank = 512 fp32)

    ctx = ExitStack()
    _cache["ctx"] = ctx
    data_sb = ctx.enter_context(nc.sbuf_tensor("data_sb", [128, STEPS * DW], mybir.dt.float32))
    idx_sb = ctx.enter_context(
        nc.sbuf_tensor("idx_sb", [128, STEPS * NCH * CW], mybir.dt.uint16)
    )
    sel_sb = ctx.enter_context(nc.sbuf_tensor("sel_sb", [128, U], mybir.dt.float32))
    NBUF = 4
    gbuf = [
        ctx.enter_context(nc.sbuf_tensor(f"gbuf{i}", [128, GW], mybir.dt.float32))
        for i in range(NBUF)
    ]
    abuf = [
        ctx.enter_context(nc.sbuf_tensor(f"abuf{i}", [128, GW], mybir.dt.float32))
        for i in range(4)
    ]
    obuf = [
        ctx.enter_context(nc.sbuf_tensor(f"obuf{i}", [U, GW], mybir.dt.float32))
        for i in range(2)
    ]
    psum = [
        ctx.enter_context(nc.psum_tensor(f"ps{i}", [U, GW], mybir.dt.float32))
        for i in range(2)
    ]
    ld_sem = ctx.enter_context(nc.semaphore("ld_sem"))
    ic_sem = ctx.enter_context(nc.semaphore("ic_sem"))
    add_sem = ctx.enter_context(nc.semaphore("add_sem"))
    mm_sem = ctx.enter_context(nc.semaphore("mm_sem"))
    cp_sem = ctx.enter_context(nc.semaphore("cp_sem"))
    dump_sem = ctx.enter_context(nc.semaphore("dump_sem"))
    block = ctx.enter_context(nc.Block())

    @block.gpsimd
    def _(gpsimd):
        gpsimd.dma_start(data_sb[:], data_d[:]).then_inc(ld_sem, 16)
        gpsimd.dma_start(idx_sb[:], idx_d[:]).then_inc(ld_sem, 16)
        gpsimd.dma_start(sel_sb[:], sel_d[:]).then_inc(ld_sem, 16)
        gpsimd.wait_ge(ld_sem, 48)
        jg = 0  # gather-tile slot counter (one per (chunk, step))
        for q in range(NCH):
            for s in range(STEPS):
                # gbuf slot reuse: PE must have consumed slot jg-NBUF.
                if jg >= NBUF:
                    gpsimd.wait_ge(mm_sem, jg - NBUF + 1)
                dst = gbuf[jg % NBUF]
                jg += 1
                dslice = data_sb[:, s * DW : (s + 1) * DW]
                ibase = s * NCH * CW + q * CW
                for sub in range(NSUB):
                    cw2 = CW // NSUB  # idx cols per sub-IC
                    isl = idx_sb[
                        :, ibase + sub * cw2 : ibase + (sub + 1) * cw2
                    ]
                    gpsimd.indirect_copy(
                        out=dst[
                            :, sub * (GW // NSUB) : (sub + 1) * (GW // NSUB)
                        ].rearrange("p (i f) -> p i f", f=F),
                        data=dslice.rearrange("p (r f) -> p r f", f=F),
                        idxs=isl,
                        i_know_ap_gather_is_preferred=True,
                    ).then_inc(ic_sem, 1)

    @block.tensor
    def _(tensor):
        # PE does the cross-step accumulation in PSUM (its own SBUF ports,
        # so the gather stream sees zero Pool-port contention from it).
        jg = 0
        for q in range(NCH):
            if q >= 2:
                tensor.wait_ge(cp_sem, q - 1)  # psum slot reused
            for s in range(STEPS):
                tensor.wait_ge(ic_sem, (q * STEPS + s + 1) * NSUB)
                for m in range(NMM):
                    mm = tensor.matmul(
                        out=psum[q % 2][:, m * 512 : (m + 1) * 512],
                        lhsT=sel_sb[:],
                        rhs=gbuf[jg % NBUF][:, m * 512 : (m + 1) * 512],
                        start=(s == 0),
                        stop=(s == STEPS - 1),
                    )
                    if m == NMM - 1:
                        mm.then_inc(mm_sem, 1)  # counts (q, s) groups
                jg += 1

    @block.scalar
    def _(scalar):
        for q in range(NCH):
            scalar.wait_ge(mm_sem, (q + 1) * STEPS)
            if q >= 2:
                scalar.wait_ge(dump_sem, (q - 1) * 16)  # obuf slot reused
            scalar.copy(obuf[q % 2][:], psum[q % 2][:]).then_inc(cp_sem, 1)

    @block.sync
    def _(sync):
        for q in range(NCH):
            sync.wait_ge(cp_sem, q + 1)
            sync.dma_start(raw_d[q], obuf[q % 2][:]).then_inc(dump_sem, 16)

    _cache["nc"] = nc
    return nc


def _install_ntff_hook():
    """Provide the antenv.axon_hooks shim the image lacks, wiring the
    ctypes NTFF profiler from trn_agent_boot."""
    import sys
    import types

    if "antenv.axon_hooks" in sys.modules:
        return
    import antenv
    from trn_agent_boot.trn_boot import _ntff_profile_via_ctypes

    mod = types.ModuleType("antenv.axon_hooks")
    hook = _ntff_profile_via_ctypes("/opt/axon/libaxon_pjrt.so")
    mod.get_axon_ntff_profile_hook = lambda: hook
    mod.set_axon_ntff_profile_hook = lambda h: None
    sys.modules["antenv.axon_hooks"] = mod
    antenv.axon_hooks = mod


def hw_exec_time_ns(trace_cores=None):
    """Re-run the last kernel() invocation with tracing; return max core ns."""
    _install_ntff_hook()
    nc = _cache["nc"]
    res = run_bass_kernel_spmd(
        nc,
        _cache["in_maps"],
        core_ids=list(range(NCORES)),
        trace=True,
        trace_cores=trace_cores,
    )
    _cache["trace"] = res
    return res.exec_time_ns


def kernel(accumulator, out_H=128, out_W=128, numangle=180, numrho=184):
    accumulator = np.asarray(accumulator, np.float32)
    assert accumulator.shape == (N, C, A, R), accumulator.shape
    assert int(out_H) == H and int(out_W) == W
    assert int(numangle) == A and int(numrho) == R

    nc = _build_nc()
    acc_flat = np.ascontiguousarray(accumulator.reshape(N * C, A, R))
    sel = _sel_matrix()
    in_maps = [
        {
            "data": _data_for_core(acc_flat, core),
            "idx": _idx_table(core),
            "sel": sel,
        }
        for core in range(NCORES)
    ]
    _cache["in_maps"] = in_maps
    res = run_bass_kernel_spmd(nc, in_maps, core_ids=list(range(NCORES)))

    # Unshard: sum the 8 per-core partials.
    # raw[q, u, i*F + f] = partial for nc = f*16+u, yx = q*CHUNK+i
    total = np.zeros((NCPC, YX), np.float64)
    for core in range(NCORES):
        raw = res.results[core]["raw"]  # [NCH, U, CHUNK*F]
        oc = raw.reshape(NCH, U, CHUNK, F).transpose(3, 1, 0, 2).reshape(NCPC, YX)
        total += oc
    return total.astype(np.float32).reshape(N, C, H, W)



# revision 3
# speedup vs baseline: 8.4847x; 8.4847x over previous
"""Trainium2 Bass kernel for the inverse deep-hough-transform gather-reduce.

out[n, c, y, x] = sum_k acc[n, c, k, rho_idx[k, y, x]]

Design (v3): one-hot matmul gather on the PE (tensor engine)
------------------------------------------------------------
For a fixed output row y and angle k, the gather over x is a selection
matmul:  out[x, nc] += sum_rho OH[rho, x] * acc_k[rho, nc], with OH the
0/1 one-hot of rho == r(k, y, x).  The PE streams the 512 nc columns at
1 col/cycle and produces 128 gathered elements per cycle - ~50x the
GPSIMD IndirectCopy rate of the previous design.

- Contraction dim K is a 128-row rho *window* per (angle, y-block): the
  per-y window never exceeds 128 rows, and its drift across a y-block of
  g rows stays within 128 for a per-angle granularity g(k) in {16,8,4,2}.
- Narrow angles (small |cos|) have windows <= 64 (or <= 32) rows even
  with 16-row drift, so 2 (or 4) of them stack into one 128-row slot:
  the matmul contracts over both, summing the angle pair in one pass.
- Sharding: angles are distributed across the 8 cores *by class* so the
  SPMD instruction stream is identical on every core; all per-core
  geometry lives in the host-built data (one-hot weight tiles + rho
  window "slab" tables).  Host sums the 8 per-core partial outputs.
- Per y: ~21 accumulating matmuls into one PSUM bank (8 banks cycle),
  ACT evicts PSUM->SBUF, sync DMAs the row to HBM.  Weight tiles and
  slab blocks stream HBM->SBUF on the gpsimd queue, double buffered.
"""

from contextlib import ExitStack

import ml_dtypes
import numpy as np

import concourse.bass as bass
from concourse import mybir
from concourse.bass_utils import run_bass_kernel_spmd

BF16 = ml_dtypes.bfloat16

# Problem constants (hardcoded per the harness contract).
N, C, A, R = 4, 128, 180, 184
H = W = 128
NC = N * C  # 512
NCORES = 8
NY = H  # output rows, one PSUM accumulation group each
NBANK = 8  # PSUM banks
NWRING = 8  # weight ring depth (y slots)
NOBUF = 4  # output staging buffers

_cache = {}


def _rho_table():
    """r[k, y, x] int32 rho index; here always in [0, R)."""
    if "r" not in _cache:
        k = np.arange(A)
        theta = k * (np.pi / A)
        cos_t, sin_t = np.cos(theta), np.sin(theta)
        y, x = np.meshgrid(np.arange(H), np.arange(W), indexing="ij")
        xc = (x - W // 2).astype(np.float64)
        yc = (y - H // 2).astype(np.float64)
        r = np.round(cos_t[:, None, None] * xc[None] + sin_t[:, None, None] * yc[None])
        r = r.astype(np.int64) + R // 2
        assert (r >= 0).all() and (r < R).all()
        _cache["r"] = r.astype(np.int32)
    return _cache["r"]


def _geometry():
    """Static geometry: per-core position plan + DMA schedule (identical
    instruction stream across cores; only data differs)."""
    if "geo" in _cache:
        return _cache["geo"]
    r = _rho_table()
    lo = r.min(axis=2)  # [A, H]
    hi = r.max(axis=2)

    def block_win(k, g):
        w = 0
        for b in range(0, NY, g):
            w = max(w, hi[k, b : b + g].max() - lo[k, b : b + g].min() + 1)
        return w

    gk = np.zeros(A, np.int32)
    for k in range(A):
        for g in (16, 8, 4, 2):
            if block_win(k, g) <= 128:
                gk[k] = g
                break
        assert gk[k] > 0, k

    # Classes: (lane rows, granularity).  g=16 singles split by window
    # width into pair/quad-packable pools.
    t128, t64, t32 = [], [], []
    for k in np.nonzero(gk == 16)[0]:
        w = block_win(k, 16)
        (t32 if w <= 32 else t64 if w <= 64 else t128).append(int(k))
    t8 = [int(k) for k in np.nonzero(gk == 8)[0]]
    t4 = [int(k) for k in np.nonzero(gk == 4)[0]]
    t2 = [int(k) for k in np.nonzero(gk == 2)[0]]

    def npos(pool, lanes):
        return -(-len(pool) // (NCORES * lanes))

    # Position profile, shared by all cores: (nlanes, lane_rows, g, pool)
    profile = (
        [(1, 128, 16)] * npos(t128, 1)
        + [(2, 64, 16)] * npos(t64, 2)
        + [(4, 32, 16)] * npos(t32, 4)
        + [(1, 128, 8)] * npos(t8, 1)
        + [(1, 128, 4)] * npos(t4, 1)
        + [(1, 128, 2)] * npos(t2, 1)
    )
    P = len(profile)

    # Assign pool angles to (core, position, lane); None = dummy (zeros).
    pools = {
        (1, 16): list(t128), (2, 16): list(t64), (4, 16): list(t32),
        (1, 8): list(t8), (1, 4): list(t4), (1, 2): list(t2),
    }
    lanes = [[[] for _ in range(P)] for _ in range(NCORES)]
    for key in pools:
        nl = key[0]
        pos_ids = [i for i, (l, _, g) in enumerate(profile) if (l, g) == key]
        slots = [
            (c, i, j) for i in pos_ids for j in range(nl) for c in range(NCORES)
        ]
        pool = pools[key]
        for sidx, (c, i, j) in enumerate(slots):
            lanes[c][i].append(pool[sidx] if sidx < len(pool) else None)
    # lanes[c][i] is ordered by lane index j.

    # Slab slots: position i has NY // g_i blocks, double buffered.
    slot_of = {}
    nslot = 0
    for i, (_, _, g) in enumerate(profile):
        for b in range(NY // g):
            slot_of[(i, b)] = nslot
            nslot += 1

    # DMA schedule sorted by deadline.  items: (kind, i, b, flow_wait)
    items = []
    for i, (_, _, g) in enumerate(profile):
        for b in range(NY // g):
            fw = (b - 1) * g if b >= 2 else None
            items.append((b * g, 0, "slab", i, b, fw))
    for y in range(NY):
        fw = y - NWRING + 1 if y >= NWRING else None
        items.append((y, 1, "wt", y, 0, fw))
    items.sort(key=lambda t: (t[0], t[1]))
    ndma_needed = np.zeros(NY, np.int64)
    cnt = 0
    ptr = 0
    for y in range(NY):
        while ptr < len(items) and items[ptr][0] <= y:
            cnt += 1
            ptr += 1
        ndma_needed[y] = cnt

    # Per-(angle, block) slab row offsets, per core (data-side only).
    # offset o: lane rows [o, o+rows) of acc_k; one-hot rows r - o.
    def lane_off(k, rows, g, b):
        l = int(lo[k, b * g : (b + 1) * g].min())
        h = int(hi[k, b * g : (b + 1) * g].max())
        assert h - l + 1 <= rows, (k, rows, g, b)
        return min(l, R - rows)

    _cache["geo"] = dict(
        profile=profile, P=P, lanes=lanes, slot_of=slot_of, nslot=nslot,
        items=items, ndma_needed=ndma_needed, lane_off=lane_off,
    )
    return _cache["geo"]


def _host_tables():
    """Per-core one-hot weight tables (geometry only; cached across calls)
    and slab assembly metadata."""
    if "wts" in _cache:
        return _cache["wts"], _cache["slab_meta"]
    geo = _geometry()
    r = _rho_table()
    P = geo["P"]
    profile = geo["profile"]
    wts = []
    slab_meta = []  # per core: list over slots of [(k, o, rows, lane_j)]
    xs = np.arange(W)
    ys = np.arange(NY)
    for c in range(NCORES):
        w = np.zeros((NY, 128, P * 128), BF16)
        meta = [[] for _ in range(geo["nslot"])]
        for i, (nl, rows, g) in enumerate(profile):
            for j, k in enumerate(geo["lanes"][c][i]):
                if k is None:
                    continue
                for b in range(NY // g):
                    o = geo["lane_off"](k, rows, g, b)
                    meta[geo["slot_of"][(i, b)]].append((k, o, rows, j))
                    yb = ys[b * g : (b + 1) * g]
                    rowidx = r[k, yb][:, :] - o + j * rows  # [g, W]
                    w[yb[:, None], rowidx, i * 128 + xs[None, :]] = 1
        wts.append(w)
        slab_meta.append(meta)
    _cache["wts"] = wts
    _cache["slab_meta"] = slab_meta
    return wts, slab_meta


def _build_nc():
    if "nc" in _cache:
        return _cache["nc"]
    geo = _geometry()
    P = geo["P"]
    nslot = geo["nslot"]
    ndma = geo["ndma_needed"]

    nc = bass.Bass("TRN2", debug=False, target_bir_lowering=False, num_devices=NCORES)
    wts_d = nc.dram_tensor(
        "wts", [NY, 128, P * 128], mybir.dt.bfloat16, kind="ExternalInput"
    ).ap()
    slab_d = nc.dram_tensor(
        "slabs", [nslot, 128, NC], mybir.dt.bfloat16, kind="ExternalInput"
    ).ap()
    out_d = nc.dram_tensor(
        "out", [NY, 128, NC], mybir.dt.float32, kind="ExternalOutput"
    ).ap()

    ctx = ExitStack()
    _cache["ctx"] = ctx
    slabs_sb = ctx.enter_context(
        nc.sbuf_tensor("slabs_sb", [128, P * 2 * NC], mybir.dt.bfloat16)
    )
    wring = ctx.enter_context(
        nc.sbuf_tensor("wring", [128, NWRING * P * 128], mybir.dt.bfloat16)
    )
    obuf = ctx.enter_context(
        nc.sbuf_tensor("obuf", [128, NOBUF * NC], mybir.dt.float32)
    )
    ps = [
        ctx.enter_context(nc.psum_tensor(f"ps{i}", [128, NC], mybir.dt.float32))
        for i in range(NBANK)
    ]
    dma_sem = ctx.enter_context(nc.semaphore("dma_sem"))
    mm_sem = ctx.enter_context(nc.semaphore("mm_sem"))
    cp_sem = ctx.enter_context(nc.semaphore("cp_sem"))
    dump_sem = ctx.enter_context(nc.semaphore("dump_sem"))
    block = ctx.enter_context(nc.Block())

    def slab_col(i, half):
        return (i * 2 + half) * NC

    @block.gpsimd
    def _(gpsimd):
        for _, _, kind, i, b, fw in geo["items"]:
            if fw is not None:
                gpsimd.wait_ge(mm_sem, fw)
            if kind == "slab":
                col = slab_col(i, b % 2)
                gpsimd.dma_start(
                    slabs_sb[:, col : col + NC], slab_d[geo["slot_of"][(i, b)]]
                ).then_inc(dma_sem, 16)
            else:
                y = i
                base = (y % NWRING) * P * 128
                gpsimd.dma_start(
                    wring[:, base : base + P * 128], wts_d[y]
                ).then_inc(dma_sem, 16)

    @block.tensor
    def _(tensor):
        for y in range(NY):
            if y >= NBANK:
                tensor.wait_ge(cp_sem, y - NBANK + 1)
            tensor.wait_ge(dma_sem, 16 * int(ndma[y]))
            wbase = (y % NWRING) * P * 128
            for i, (_, _, g) in enumerate(geo["profile"]):
                half = (y // g) % 2
                col = slab_col(i, half)
                mm = tensor.matmul(
                    out=ps[y % NBANK][:],
                    lhsT=wring[:, wbase + i * 128 : wbase + (i + 1) * 128],
                    rhs=slabs_sb[:, col : col + NC],
                    start=(i == 0),
                    stop=(i == P - 1),
                )
            mm.then_inc(mm_sem, 1)

    @block.scalar
    def _(scalar):
        for y in range(NY):
            scalar.wait_ge(mm_sem, y + 1)
            if y >= NOBUF:
                scalar.wait_ge(dump_sem, 16 * (y - NOBUF + 1))
            col = (y % NOBUF) * NC
            scalar.copy(obuf[:, col : col + NC], ps[y % NBANK][:]).then_inc(cp_sem, 1)

    @block.sync
    def _(sync):
        for y in range(NY):
            sync.wait_ge(cp_sem, y + 1)
            col = (y % NOBUF) * NC
            sync.dma_start(out_d[y], obuf[:, col : col + NC]).then_inc(dump_sem, 16)

    _cache["nc"] = nc
    return nc


def _install_ntff_hook():
    """Provide the antenv.axon_hooks shim the image lacks, wiring the
    ctypes NTFF profiler from trn_agent_boot."""
    import sys
    import types

    if "antenv.axon_hooks" in sys.modules:
        return
    import antenv
    from trn_agent_boot.trn_boot import _ntff_profile_via_ctypes

    mod = types.ModuleType("antenv.axon_hooks")
    hook = _ntff_profile_via_ctypes("/opt/axon/libaxon_pjrt.so")
    mod.get_axon_ntff_profile_hook = lambda: hook
    mod.set_axon_ntff_profile_hook = lambda h: None
    sys.modules["antenv.axon_hooks"] = mod
    antenv.axon_hooks = mod


def hw_exec_time_ns(trace_cores=None):
    """Re-run the last kernel() invocation with tracing; return max core ns."""
    _install_ntff_hook()
    nc = _cache["nc"]
    res = run_bass_kernel_spmd(
        nc,
        _cache["in_maps"],
        core_ids=list(range(NCORES)),
        trace=True,
        trace_cores=trace_cores,
    )
    _cache["trace"] = res
    return res.exec_time_ns


def kernel(accumulator, out_H=128, out_W=128, numangle=180, numrho=184):
    accumulator = np.asarray(accumulator, np.float32)
    assert accumulator.shape == (N, C, A, R), accumulator.shape
    assert int(out_H) == H and int(out_W) == W
    assert int(numangle) == A and int(numrho) == R

    geo = _geometry()
    wts, slab_meta = _host_tables()
    nc = _build_nc()

    # acc_t[k, rho, nc] bf16 - slab source.
    acc_t = np.ascontiguousarray(
        accumulator.reshape(NC, A, R).transpose(1, 2, 0)
    ).astype(BF16)

    in_maps = []
    for c in range(NCORES):
        slabs = np.zeros((geo["nslot"], 128, NC), BF16)
        for slot, entries in enumerate(slab_meta[c]):
            for k, o, rows, j in entries:
                slabs[slot, j * rows : (j + 1) * rows] = acc_t[k, o : o + rows]
        in_maps.append({"wts": wts[c], "slabs": slabs})
    _cache["in_maps"] = in_maps
    res = run_bass_kernel_spmd(nc, in_maps, core_ids=list(range(NCORES)))

    # Unshard: sum the 8 per-core partials.  out[y, x, nc]
    total = np.zeros((NY, 128, NC), np.float64)
    for c in range(NCORES):
        total += res.results[c]["out"]
    return (
        total.transpose(2, 0, 1).reshape(N, C, H, W).astype(np.float32)
    )


# revision 4
# speedup vs baseline: 8.7420x; 1.0303x over previous
"""Trainium2 Bass kernel for the inverse deep-hough-transform gather-reduce.

out[n, c, y, x] = sum_k acc[n, c, k, rho_idx[k, y, x]]

Design (v4): one-hot matmul gather on the PE (tensor engine)
------------------------------------------------------------
For a fixed output row y and angle k, the gather over x is a selection
matmul:  out[x, nc] += sum_rho OH[rho, x] * acc_k[rho, nc], with OH the
0/1 one-hot of rho == r(k, y, x).  The PE streams the 512 nc columns at
1 col/cycle and produces >= 128 gathered elements per cycle.

- Contraction dim K packs multiple angles' rho *windows* (bin packing):
  angle k needs a window of win_g(k) rho rows covering a y-block of g(k)
  rows (g in {16,8,4,2} per angle; finer g for |cos| ~ 1 angles whose
  window drifts fast with y).  First-fit-decreasing packs the windows
  into 128-row bins; one bin = one matmul per y, summing all its angles.
- Sharding: bins are dealt across the 8 cores class-by-class so the SPMD
  instruction stream is identical on every core; all per-core geometry
  lives in host-built data (one-hot weight tiles + rho window "slabs").
  Host sums the 8 per-core partial outputs.
- Per y: P (~18) accumulating matmuls into one PSUM bank (8 banks
  cycle), ACT evicts PSUM->SBUF, sync DMAs the row out to HBM.  Weight
  tiles and slab blocks stream HBM->SBUF on the gpsimd queue.
- Sync uses one semaphore per SBUF slot (weight-ring slot / slab
  double-buffer half) so correctness does not depend on cross-DMA
  completion ordering: successive DMAs into the *same* slot are already
  serialized by the consumption flow control.
"""

from contextlib import ExitStack

import ml_dtypes
import numpy as np

import concourse.bass as bass
from concourse import mybir
from concourse.bass_utils import run_bass_kernel_spmd

BF16 = ml_dtypes.bfloat16

# Problem constants (hardcoded per the harness contract).
N, C, A, R = 4, 128, 180, 184
H = W = 128
NC = N * C  # 512
NCORES = 8
NY = H  # output rows, one PSUM accumulation group each
NBANK = 8  # PSUM banks
NWRING = 8  # weight ring depth (y slots)
NOBUF = 4  # output staging buffers

_cache = {}


def _rho_table():
    """r[k, y, x] int32 rho index; always in [0, R) for this geometry."""
    if "r" not in _cache:
        k = np.arange(A)
        theta = k * (np.pi / A)
        cos_t, sin_t = np.cos(theta), np.sin(theta)
        y, x = np.meshgrid(np.arange(H), np.arange(W), indexing="ij")
        xc = (x - W // 2).astype(np.float64)
        yc = (y - H // 2).astype(np.float64)
        r = np.round(cos_t[:, None, None] * xc[None] + sin_t[:, None, None] * yc[None])
        r = r.astype(np.int64) + R // 2
        assert (r >= 0).all() and (r < R).all()
        _cache["r"] = r.astype(np.int32)
    return _cache["r"]


def _geometry():
    """Static geometry: per-core bin plan + DMA schedule (instruction
    stream identical across cores; only data differs)."""
    if "geo" in _cache:
        return _cache["geo"]
    r = _rho_table()
    lo = r.min(axis=2)  # [A, H]
    hi = r.max(axis=2)

    def win_at_g(k, g):
        w = 0
        for b in range(0, NY, g):
            w = max(w, int(hi[k, b : b + g].max() - lo[k, b : b + g].min()) + 1)
        return w

    gk = {}
    for k in range(A):
        for g in (16, 8, 4, 2):
            if win_at_g(k, g) <= 128:
                gk[k] = g
                break
        assert k in gk

    # FFD bin packing per granularity class.
    def ffd(items):
        bins = []
        for w, k in sorted(items, reverse=True):
            for b in bins:
                if b[0] + w <= 128:
                    b[0] += w
                    b[1].append((k, w))
                    break
            else:
                bins.append([w, [(k, w)]])
        return [b[1] for b in bins]

    core_bins = [[] for _ in range(NCORES)]  # per core: list of (g, lanes)
    profile = []  # per position: g  (shared across cores)
    for g in (16, 8, 4, 2):
        items = [(win_at_g(k, g), k) for k in range(A) if gk[k] == g]
        if not items:
            continue
        bins = ffd(items)
        npos = -(-len(bins) // NCORES)
        profile += [g] * npos
        for i in range(npos * NCORES):
            core_bins[i % NCORES].append(
                (g, bins[i] if i < len(bins) else [])
            )
    P = len(profile)

    # lanes[c][i] = list of (k, width, base_row); bases are prefix sums.
    lanes = [[] for _ in range(NCORES)]
    for c in range(NCORES):
        for g, lane_list in core_bins[c]:
            out, base = [], 0
            for k, w in lane_list:
                out.append((k, w, base))
                base += w
            assert base <= 128
            lanes[c].append(out)

    # Slab slots: position i has NY // g_i blocks, double buffered.
    slot_of = {}
    nslot = 0
    for i, g in enumerate(profile):
        for b in range(NY // g):
            slot_of[(i, b)] = nslot
            nslot += 1

    # DMA schedule sorted by deadline: (deadline, tie, kind, i, b, flow_wait)
    items = []
    for i, g in enumerate(profile):
        for b in range(NY // g):
            fw = (b - 1) * g if b >= 2 else None
            items.append((b * g, 0, "slab", i, b, fw))
    for y in range(NY):
        fw = y - NWRING + 1 if y >= NWRING else None
        items.append((y, 1, "wt", y, 0, fw))
    items.sort(key=lambda t: (t[0], t[1]))

    def lane_off(k, width, g, b):
        l = int(lo[k, b * g : (b + 1) * g].min())
        h = int(hi[k, b * g : (b + 1) * g].max())
        assert h - l + 1 <= width
        return min(l, R - width)

    _cache["geo"] = dict(
        profile=profile, P=P, lanes=lanes, slot_of=slot_of, nslot=nslot,
        items=items, lane_off=lane_off,
    )
    return _cache["geo"]


def _host_tables():
    """Per-core one-hot weight tables (geometry only; cached across calls)
    and slab assembly metadata."""
    if "wts" in _cache:
        return _cache["wts"], _cache["slab_meta"]
    geo = _geometry()
    r = _rho_table()
    P = geo["P"]
    profile = geo["profile"]
    wts = []
    slab_meta = []  # per core: list over slots of [(k, o, width, base)]
    xs = np.arange(W)
    ys = np.arange(NY)
    for c in range(NCORES):
        w = np.zeros((NY, 128, P * 128), BF16)
        meta = [[] for _ in range(geo["nslot"])]
        for i, g in enumerate(profile):
            for k, width, base in geo["lanes"][c][i]:
                for b in range(NY // g):
                    o = geo["lane_off"](k, width, g, b)
                    meta[geo["slot_of"][(i, b)]].append((k, o, width, base))
                    yb = ys[b * g : (b + 1) * g]
                    rowidx = r[k, yb] - o + base  # [g, W]
                    w[yb[:, None], rowidx, i * 128 + xs[None, :]] = 1
        wts.append(w)
        slab_meta.append(meta)
    _cache["wts"] = wts
    _cache["slab_meta"] = slab_meta
    return wts, slab_meta


def _build_nc():
    if "nc" in _cache:
        return _cache["nc"]
    geo = _geometry()
    P = geo["P"]
    profile = geo["profile"]
    nslot = geo["nslot"]

    nc = bass.Bass("TRN2", debug=False, target_bir_lowering=False, num_devices=NCORES)
    wts_d = nc.dram_tensor(
        "wts", [NY, 128, P * 128], mybir.dt.bfloat16, kind="ExternalInput"
    ).ap()
    slab_d = nc.dram_tensor(
        "slabs", [nslot, 128, NC], mybir.dt.bfloat16, kind="ExternalInput"
    ).ap()
    out_d = nc.dram_tensor(
        "out", [NY, 128, NC], mybir.dt.float32, kind="ExternalOutput"
    ).ap()

    ctx = ExitStack()
    _cache["ctx"] = ctx
    slabs_sb = ctx.enter_context(
        nc.sbuf_tensor("slabs_sb", [128, P * 2 * NC], mybir.dt.bfloat16)
    )
    wring = ctx.enter_context(
        nc.sbuf_tensor("wring", [128, NWRING * P * 128], mybir.dt.bfloat16)
    )
    obuf = ctx.enter_context(
        nc.sbuf_tensor("obuf", [128, NOBUF * NC], mybir.dt.float32)
    )
    ps = [
        ctx.enter_context(nc.psum_tensor(f"ps{i}", [128, NC], mybir.dt.float32))
        for i in range(NBANK)
    ]
    mm_sem = ctx.enter_context(nc.semaphore("mm_sem"))
    cp_sem = ctx.enter_context(nc.semaphore("cp_sem"))
    dump_sem = ctx.enter_context(nc.semaphore("dump_sem"))
    wt_sems = [
        ctx.enter_context(nc.semaphore(f"wt{s}")) for s in range(NWRING)
    ]
    sl_sems = [
        [ctx.enter_context(nc.semaphore(f"sl{i}_{h}")) for h in range(2)]
        for i in range(P)
    ]
    block = ctx.enter_context(nc.Block())

    def slab_col(i, half):
        return (i * 2 + half) * NC

    @block.gpsimd
    def _(gpsimd):
        for _, _, kind, i, b, fw in geo["items"]:
            if fw is not None:
                gpsimd.wait_ge(mm_sem, fw)
            if kind == "slab":
                col = slab_col(i, b % 2)
                gpsimd.dma_start(
                    slabs_sb[:, col : col + NC], slab_d[geo["slot_of"][(i, b)]]
                ).then_inc(sl_sems[i][b % 2], 16)
            else:
                y = i
                base = (y % NWRING) * P * 128
                gpsimd.dma_start(
                    wring[:, base : base + P * 128], wts_d[y]
                ).then_inc(wt_sems[y % NWRING], 16)

    @block.tensor
    def _(tensor):
        for y in range(NY):
            if y >= NBANK:
                tensor.wait_ge(cp_sem, y - NBANK + 1)
            tensor.wait_ge(wt_sems[y % NWRING], 16 * (y // NWRING + 1))
            for i, g in enumerate(profile):
                if y % g == 0:
                    b = y // g
                    tensor.wait_ge(sl_sems[i][b % 2], 16 * (b // 2 + 1))
            wbase = (y % NWRING) * P * 128
            for i, g in enumerate(profile):
                col = slab_col(i, (y // g) % 2)
                mm = tensor.matmul(
                    out=ps[y % NBANK][:],
                    lhsT=wring[:, wbase + i * 128 : wbase + (i + 1) * 128],
                    rhs=slabs_sb[:, col : col + NC],
                    start=(i == 0),
                    stop=(i == P - 1),
                )
            mm.then_inc(mm_sem, 1)

    @block.scalar
    def _(scalar):
        for y in range(NY):
            scalar.wait_ge(mm_sem, y + 1)
            if y >= NOBUF:
                scalar.wait_ge(dump_sem, 16 * (y - NOBUF + 1))
            col = (y % NOBUF) * NC
            scalar.copy(obuf[:, col : col + NC], ps[y % NBANK][:]).then_inc(cp_sem, 1)

    @block.sync
    def _(sync):
        for y in range(NY):
            sync.wait_ge(cp_sem, y + 1)
            col = (y % NOBUF) * NC
            sync.dma_start(out_d[y], obuf[:, col : col + NC]).then_inc(dump_sem, 16)

    _cache["nc"] = nc
    return nc


def _install_ntff_hook():
    """Provide the antenv.axon_hooks shim the image lacks, wiring the
    ctypes NTFF profiler from trn_agent_boot."""
    import sys
    import types

    if "antenv.axon_hooks" in sys.modules:
        return
    import antenv
    from trn_agent_boot.trn_boot import _ntff_profile_via_ctypes

    mod = types.ModuleType("antenv.axon_hooks")
    hook = _ntff_profile_via_ctypes("/opt/axon/libaxon_pjrt.so")
    mod.get_axon_ntff_profile_hook = lambda: hook
    mod.set_axon_ntff_profile_hook = lambda h: None
    sys.modules["antenv.axon_hooks"] = mod
    antenv.axon_hooks = mod


def hw_exec_time_ns(trace_cores=None):
    """Re-run the last kernel() invocation with tracing; return max core ns."""
    _install_ntff_hook()
    nc = _cache["nc"]
    res = run_bass_kernel_spmd(
        nc,
        _cache["in_maps"],
        core_ids=list(range(NCORES)),
        trace=True,
        trace_cores=trace_cores,
    )
    _cache["trace"] = res
    return res.exec_time_ns


def kernel(accumulator, out_H=128, out_W=128, numangle=180, numrho=184):
    accumulator = np.asarray(accumulator, np.float32)
    assert accumulator.shape == (N, C, A, R), accumulator.shape
    assert int(out_H) == H and int(out_W) == W
    assert int(numangle) == A and int(numrho) == R

    geo = _geometry()
    wts, slab_meta = _host_tables()
    nc = _build_nc()

    # acc_t[k, rho, nc] bf16 - slab source.
    acc_t = np.ascontiguousarray(
        accumulator.reshape(NC, A, R).transpose(1, 2, 0)
    ).astype(BF16)

    in_maps = []
    for c in range(NCORES):
        slabs = np.zeros((geo["nslot"], 128, NC), BF16)
        for slot, entries in enumerate(slab_meta[c]):
            for k, o, width, base in entries:
                slabs[slot, base : base + width] = acc_t[k, o : o + width]
        in_maps.append({"wts": wts[c], "slabs": slabs})
    _cache["in_maps"] = in_maps
    res = run_bass_kernel_spmd(nc, in_maps, core_ids=list(range(NCORES)))

    # Unshard: sum the 8 per-core partials.  out[y, x, nc]
    total = np.zeros((NY, 128, NC), np.float64)
    for c in range(NCORES):
        total += res.results[c]["out"]
    return (
        total.transpose(2, 0, 1).reshape(N, C, H, W).astype(np.float32)
    )
